# revision 5
# baseline (speedup 1.0000x reference)
"""Trainium2 Bass kernel v2 for nn_DecoupledEmbeddingModel
(B=2048, D=512, C=110, H=1024, V=50257), 8 NeuronCores.

Strategy:
- Front (embed->MLP stack->final LN) stays data-parallel: 256 rows/core,
  bf16 matmuls, PE transposes, Rsqrt-only scalar-engine usage.
- Final activations are split into an fp8e4 (value, residual) pair,
  transposed, packed into one buffer per row tile and AllGathered (one
  collective per row tile so the first can launch early).
- lm_head is vocab-sharded: each core computes all 2048 rows against its
  6336-col vocab slice using fp8 DoubleRow matmuls, 3 passes
  (A8@W8 + R8@W8 + A8@S8) for bf16-level accuracy.
- Output [2048, 6336] bf16 per core holds SA*SW-scaled logits; the host
  descales (exact power of two) and concatenates along vocab.
"""

import math
import sys

sys.path.insert(0, "/opt/trn_rl_repo")

import numpy as np
import ml_dtypes

import concourse.bass as bass
import concourse.tile as tile
from concourse import bacc, mybir
from concourse.bass_utils import run_bass_kernel_spmd
from concourse.masks import make_identity

AF = mybir.ActivationFunctionType
ALU = mybir.AluOpType
DR = mybir.MatmulPerfMode.DoubleRow
f32 = mybir.dt.float32
bf16 = mybir.dt.bfloat16
fp8 = mybir.dt.float8e4
P = 128

# Model dims
V, D, C, H = 50257, 512, 110, 1024
B = 2048
NCORES = 8

BB = B // NCORES          # 256 rows/core
RT_N = BB // P            # 2 row tiles/core
VT = 512
VC = 6336                 # vocab cols per core (8*6336 = 50688 = pad(V))
NV = 13                   # 12 full 512-tiles + one 192-tile
VW = [512] * 12 + [192]
K2 = 3                    # vocab tiles per phase computed with 2 fp8 passes
SA = 8.0                  # activation pre-quant scale (folded into final LN)
SW = 64.0                 # weight pre-quant scale
DESCALE = 1.0 / (SA * SW)

e4m3 = ml_dtypes.float8_e4m3


def np_dt(dt):
    return np.dtype(mybir.dt.np(dt))


# ----------------------------------------------------------------------------
# Device program
# ----------------------------------------------------------------------------

def build_nc(sim=False, lm_passes=3, fast=True):
    """sim=True replaces collectives with local copies so CoreSim can run
    core 0 standalone (remote row tiles then read zeros)."""
    nc = bacc.Bacc("TRN2", target_bir_lowering=False, debug=False,
                   enable_asserts=False, num_devices=NCORES)

    a = {}
    def din(name, shape, dt=f32):
        a[name] = nc.dram_tensor(name, list(shape), dt, kind="ExternalInput").ap()

    din("tok", [BB, 1], mybir.dt.int32)
    din("etab", [V, D])
    din("cp_w", [P, 4, C], bf16)
    din("cp_b", [C])
    din("h1_w", [P, H], bf16)       # rows 0..109 = h_w1.T, 110 = h_b1, rest 0
    din("h2_w", [P, 8, C], bf16)
    din("h2_b", [C])
    din("up_w", [P, D], bf16)       # rows 0..109 = up_w.T, 110 = up_b, rest 0
    din("rn_g", [D]); din("rn_b", [D])
    din("p1_w", [P, 16, H], bf16)   # fusion-LN g/b folded in
    din("p1_b", [H])
    din("p2_w", [P, 8, D], bf16)    # pln-LN g/b folded in
    din("p2_b", [D])
    # W8 and S8 residual: [vt, k, (w|s), b, i, n], d = (2b+i)*128 + k
    din("wvs8", [NV, P, 2, 2, 2, VT], fp8)
    out_ap = nc.dram_tensor("out", [B, VC], bf16, kind="ExternalOutput").ap()

    # Collective bounce buffers (partition-major payload so both the bounce
    # write and the gather read run at full DMA rate)
    agin = [nc.dram_tensor(f"agin{r}", [P, 2, 4, P], fp8).ap()
            for r in range(RT_N)]
    agout = [nc.dram_tensor(f"agout{r}", [NCORES, P, 2, 4, P], fp8).ap()
             for r in range(RT_N)]

    with tile.TileContext(nc) as tc:
        _program(tc, a, out_ap, agin, agout, sim, lm_passes, fast)
    nc.compile()
    return nc


def _program(tc, a, out_ap, agin, agout, sim, lm_passes, fast):
    nc = tc.nc
    from contextlib import ExitStack
    ctx = ExitStack()
    with ctx:
        consts = ctx.enter_context(tc.tile_pool(name="consts", bufs=1))
        workA = ctx.enter_context(tc.tile_pool(name="workA", bufs=2))
        workB = ctx.enter_context(tc.tile_pool(name="workB", bufs=2))
        tpool = ctx.enter_context(tc.tile_pool(name="tpool", bufs=2))
        lmw = ctx.enter_context(tc.tile_pool(name="lmw", bufs=4))
        lms = ctx.enter_context(tc.tile_pool(name="lms", bufs=2))
        ps_acc = ctx.enter_context(tc.tile_pool(name="ps_acc", bufs=2, space="PSUM"))
        ps_t = ctx.enter_context(tc.tile_pool(name="ps_t", bufs=2, space="PSUM"))
        ps_lm = ctx.enter_context(tc.tile_pool(name="ps_lm", bufs=4, space="PSUM"))

        ident = consts.tile([P, P], f32)
        make_identity(nc, ident[:])
        eps5 = consts.tile([P, 1], f32)
        nc.vector.memset(eps5[:], 1e-5)
        eps24 = consts.tile([P, 1], f32)
        nc.vector.memset(eps24[:], 1e-24)

        # ---------------- front (generator; two row tiles interleaved) -----
        # emb gathers first so neither chain's head is queue-blocked
        tokts, embs = [], []
        for rt in range(RT_N):
            tokt = workB.tile([P, 1], mybir.dt.int32, tag="tok")
            nc.sync.dma_start(out=tokt[:], in_=a["tok"][rt * P:(rt + 1) * P, :])
            tokts.append(tokt)
        for rt in range(RT_N):
            emb = workB.tile([P, D], f32, tag="emb")
            nc.gpsimd.indirect_dma_start(
                out=emb[:], out_offset=None,
                in_=a["etab"][:],
                in_offset=bass.IndirectOffsetOnAxis(ap=tokts[rt][:, :1], axis=0),
            )
            embs.append(emb)

        def brep(name, n):
            t = consts.tile([P, n], f32, tag=f"br_{name}")
            src = a[name]
            bsrc = bass.AP(tensor=src.tensor, offset=src.offset,
                           ap=[[0, P]] + list(src.ap))
            nc.gpsimd.dma_start(out=t[:], in_=bsrc)
            return t

        # broadcast-bias loads AFTER the emb gathers (Pool queue order) so the
        # gathers' descriptor generation and DMA slots go first
        if fast:
            cp_b = h2_b = rn_g = rn_b = p1_b = p2_b = None
        else:
            cp_b = brep("cp_b", C)
            h2_b = brep("h2_b", C)
            rn_g = brep("rn_g", D); rn_b = brep("rn_b", D)
            p1_b = brep("p1_b", H)
            p2_b = brep("p2_b", D)

        def wload(name, shape, dt=bf16):
            t = consts.tile(list(shape), dt, tag=f"w_{name}")
            nc.sync.dma_start(out=t[:], in_=a[name])
            return t

        cp_w = wload("cp_w", [P, 4, C])
        h1_w = wload("h1_w", [P, H])
        h2_w = wload("h2_w", [P, 8, C])
        up_w = wload("up_w", [P, D])
        p1_w = consts.tile([P, 16, H], bf16, tag="w_p1_w")
        p2_w = consts.tile([P, 8, D], bf16, tag="w_p2_w")
        big_loaded = [False]

        def load_big_weights():
            if not big_loaded[0]:
                big_loaded[0] = True
                nc.gpsimd.dma_start(out=p1_w[:], in_=a["p1_w"])
                nc.gpsimd.dma_start(out=p2_w[:], in_=a["p2_w"])

        # fp8 (value|residual) pack per own row tile; the AG payload
        arpack = [consts.tile([P, 2, 4, P], fp8, tag=f"arp{r}", name=f"arp{r}")
                  for r in range(RT_N)]

        # cin/cout tiles pre-allocated so pad columns are zeroed off-chain
        cins = [workB.tile([P, P], f32, tag="cin", name=f"cin{r}")
                for r in range(RT_N)]
        couts = [workB.tile([P, P], f32, tag="cout", name=f"cout{r}")
                 for r in range(RT_N)]
        for r in range(RT_N):
            nc.gpsimd.memset(cins[r][:, C + 1:], 0.0)
            nc.gpsimd.memset(couts[r][:, C + 1:], 0.0)

        # ---------------- helpers ----------------
        def inv_norm(x, n):
            """Return [P,1] tile = 1/max(||x row||, eps) via bn_stats."""
            nsub = max(1, n // 512)
            st = workB.tile([P, nsub, 6], f32, tag="l2st")
            for i in range(nsub):
                s, e = i * 512, min(n, (i + 1) * 512)
                nc.vector.bn_stats(out=st[:, i, :], in_=x[:, s:e])
            mv = workB.tile([P, 2], f32, tag="l2mv")
            nc.vector.bn_aggr(out=mv[:], in_=st[:])
            ss = workB.tile([P, 1], f32, tag="ss")
            nc.vector.tensor_scalar(ss[:], mv[:, 0:1], mv[:, 0:1], None,
                                    op0=ALU.mult)
            nc.vector.tensor_add(ss[:], ss[:], mv[:, 1:2])
            # ss = sumsq/n ; 1/||x|| = 1/sqrt(n*ss)
            nc.scalar.activation(out=ss[:], in_=ss[:], func=AF.Sqrt,
                                 scale=float(n), bias=eps24[:])
            nc.vector.reciprocal(ss[:], ss[:])
            return ss

        def layernorm_inplace(x, n, g=None, b=None, out_scale=None,
                              l2_merge=False, se=None):
            """x <- LN(x) (or LN(l2norm(x)) exactly when l2_merge, via
            eps-rescaling). Wide stats/apply split across DVE and Pool."""
            nsub = max(1, n // 512)
            st = workB.tile([P, nsub, 6], f32, tag="lnst")
            for i in range(nsub):
                nc.vector.bn_stats(out=st[:, i, :], in_=x[:, i * 512:(i + 1) * 512])
            mv = workB.tile([P, 2], f32, tag="lnmv")
            nc.vector.bn_aggr(out=mv[:], in_=st[:])
            if l2_merge:
                # LN(l2n(x)) = (x-m)/sqrt(v + ||x||^2 * 1e-5), ||x||^2=n(m^2+v)
                t1 = workB.tile([P, 1], f32, tag="lnt1")
                nc.vector.tensor_scalar(t1[:], mv[:, 0:1], mv[:, 0:1], None,
                                        op0=ALU.mult)
                nc.vector.tensor_add(t1[:], t1[:], mv[:, 1:2])
                nc.vector.tensor_scalar(mv[:, 1:2], t1[:], float(n) * 1e-5,
                                        mv[:, 1:2], op0=ALU.mult, op1=ALU.add)
                nc.scalar.activation(out=mv[:, 1:2], in_=mv[:, 1:2],
                                     func=AF.Sqrt)
            else:
                nc.scalar.activation(out=mv[:, 1:2], in_=mv[:, 1:2],
                                     func=AF.Sqrt, bias=eps5[:])
            nc.vector.reciprocal(mv[:, 1:2], mv[:, 1:2])
            if out_scale is not None:
                nc.vector.tensor_scalar_mul(mv[:, 1:2], mv[:, 1:2], out_scale)
            for i in range(nsub):
                if se is not None:
                    eng = se
                else:
                    eng = nc.vector if i % 2 == 0 else nc.gpsimd
                eng.tensor_scalar(x[:, i * 512:(i + 1) * 512],
                                  x[:, i * 512:(i + 1) * 512],
                                  mv[:, 0:1], mv[:, 1:2],
                                  op0=ALU.subtract, op1=ALU.mult)
            if g is not None:
                (se or nc.vector).tensor_mul(x, x, g[:])
            if b is not None:
                (se or nc.vector).tensor_add(x, x, b[:])

        tcnt = [0]

        def pe_transpose(src, kn, dst):
            """PE-transpose kn [128,128] fp32 blocks of src into dst[:, k, :]
            (dst bf16 SBUF). Blocks are grouped 4-per-PSUM-bank (accumulate
            onto pending-zero) so each group needs only ONE copy-out."""
            for g0 in range(0, kn, 4):
                m = min(4, kn - g0)
                tp = ps_t.tile([P, 4, P], f32, tag="tp")
                for j in range(m):
                    nc.tensor.matmul(
                        tp[:, j, :], lhsT=src[:, (g0 + j) * P:(g0 + j + 1) * P],
                        rhs=ident[:], is_transpose=True,
                        start=(j == 0), stop=(j == m - 1),
                        skip_group_check=True)
                tcnt[0] += 1
                if tcnt[0] % 2 == 0:
                    if kn > 1:
                        nc.scalar.activation(out=dst[:, g0:g0 + m, :],
                                             in_=tp[:, :m, :], func=AF.Copy)
                    else:
                        nc.scalar.activation(out=dst[:], in_=tp[:, 0, :],
                                             func=AF.Copy)
                else:
                    if kn > 1:
                        nc.vector.tensor_copy(out=dst[:, g0:g0 + m, :],
                                              in_=tp[:, :m, :])
                    else:
                        nc.vector.tensor_copy(out=dst[:], in_=tp[:, 0, :])

        def pad_cols(x):
            nc.vector.memset(x[:, C:C + 1], 1.0)
            nc.vector.memset(x[:, C + 1:], 0.0)

        def front(rt):
            se = None
            emb = embs[rt]
            # transposes straight from the raw gather; 1/||emb|| in parallel
            embT = tpool.tile([P, 4, P], bf16, tag="embT")
            pe_transpose(emb[:], 4, embT)
            ss_emb = inv_norm(emb[:], D)
            emb_n = workB.tile([P, D], f32, tag="emb_n")
            nc.gpsimd.tensor_scalar_mul(emb_n[:], emb[:], ss_emb[:])
            yield

            load_big_weights()
            # core_in = l2norm((emb @ cp_w.T)/||emb|| + cp_b)
            ci_ps = ps_acc.tile([P, 512], f32, tag="acc")
            for k in range(4):
                nc.tensor.matmul(ci_ps[:, :C], lhsT=embT[:, k, :],
                                 rhs=cp_w[:, k, :], start=(k == 0), stop=(k == 3))
            cin = cins[rt]
            if rt == 1:
                nc.scalar.activation(out=cin[:, :C], in_=ci_ps[:, :C],
                                     func=AF.Copy, scale=ss_emb[:])
            else:
                nc.vector.tensor_scalar_mul(cin[:, :C], ci_ps[:, :C], ss_emb[:])
            if not fast:
                nc.vector.tensor_add(cin[:, :C], cin[:, :C], cp_b[:])
            # s = ||core_in_raw||; col C carries s so bias rows stay exact
            sq = workB.tile([P, C], f32, tag="sqC")
            s_ci = workB.tile([P, 1], f32, tag="s_ci")
            nc.scalar.activation(out=sq[:], in_=cin[:, :C], func=AF.Square,
                                 accum_out=s_ci[:])
            nc.scalar.activation(out=s_ci[:], in_=s_ci[:], func=AF.Sqrt,
                                 bias=eps24[:])
            inv_ci = workB.tile([P, 1], f32, tag="inv_ci")
            nc.vector.reciprocal(inv_ci[:], s_ci[:])
            nc.vector.tensor_copy(out=cin[:, C:C + 1], in_=s_ci[:])
            cinT = tpool.tile([P, P], bf16, tag="cinT")
            pe_transpose(cin[:], 1, cinT)
            yield

            # h1 = relu(core_in @ h_w1.T + h_b1)
            h1 = workA.tile([P, H], f32, tag="h1")
            for half in range(2):
                hp = ps_acc.tile([P, 512], f32, tag="acc")
                nc.tensor.matmul(hp[:], lhsT=cinT[:],
                                 rhs=h1_w[:, half * 512:(half + 1) * 512],
                                 start=True, stop=True)
                nc.scalar.activation(out=h1[:, half * 512:(half + 1) * 512],
                                     in_=hp[:], func=AF.Relu, scale=inv_ci[:])
            h1T = tpool.tile([P, 8, P], bf16, tag="h1T")
            pe_transpose(h1[:], 8, h1T)
            yield

            # core_out = l2norm(h1 @ h_w2.T + h_b2)
            co_ps = ps_acc.tile([P, 512], f32, tag="acc")
            for k in range(8):
                nc.tensor.matmul(co_ps[:, :C], lhsT=h1T[:, k, :],
                                 rhs=h2_w[:, k, :], start=(k == 0), stop=(k == 7))
            cout = couts[rt]
            if fast:
                if rt == 1:
                    nc.scalar.activation(out=cout[:, :C], in_=co_ps[:, :C],
                                         func=AF.Copy)
                else:
                    nc.vector.tensor_copy(out=cout[:, :C], in_=co_ps[:, :C])
            else:
                nc.vector.tensor_add(out=cout[:, :C], in0=co_ps[:, :C],
                                     in1=h2_b[:])
            sq2 = workB.tile([P, C], f32, tag="sqC2")
            s_co = workB.tile([P, 1], f32, tag="s_co")
            nc.scalar.activation(out=sq2[:], in_=cout[:, :C], func=AF.Square,
                                 accum_out=s_co[:])
            nc.scalar.activation(out=s_co[:], in_=s_co[:], func=AF.Sqrt,
                                 bias=eps24[:])
            inv_co = workB.tile([P, 1], f32, tag="inv_co")
            nc.vector.reciprocal(inv_co[:], s_co[:])
            nc.vector.tensor_copy(out=cout[:, C:C + 1], in_=s_co[:])
            coutT = tpool.tile([P, P], bf16, tag="coutT")
            pe_transpose(cout[:], 1, coutT)
            yield

            # h_t = LN(l2n(core_out @ up_w.T + up_b))  (merged via eps trick)
            cu_ps = ps_acc.tile([P, 512], f32, tag="acc")
            nc.tensor.matmul(cu_ps[:], lhsT=coutT[:], rhs=up_w[:],
                             start=True, stop=True)
            ht = workB.tile([P, D], f32, tag="ht")
            nc.scalar.activation(out=ht[:], in_=cu_ps[:], func=AF.Copy,
                                 scale=inv_co[:])
            layernorm_inplace(ht[:], D, rn_g, rn_b, l2_merge=True, se=se)
            # (rn_g/rn_b are None in fast mode: pure LN)
            yield

            # fused = LN(concat([emb_n, ht, emb_n*ht, emb_n-ht]))
            fused = workA.tile([P, 4 * D], f32, tag="fused")
            nc.gpsimd.tensor_copy(out=fused[:, 0:D], in_=emb_n[:])
            nc.gpsimd.tensor_copy(out=fused[:, D:2 * D], in_=ht[:])
            nc.gpsimd.tensor_mul(out=fused[:, 2 * D:3 * D], in0=emb_n[:],
                                 in1=ht[:])
            nc.gpsimd.tensor_sub(out=fused[:, 3 * D:4 * D], in0=emb_n[:],
                                 in1=ht[:])
            layernorm_inplace(fused[:], 4 * D, se=se)
            yield

            fusedT = tpool.tile([P, 16, P], bf16, tag="fusedT")
            pe_transpose(fused[:], 16, fusedT)
            yield

            # x = LN(gelu(fused @ p1_w.T + p1_b))
            x1a = ps_acc.tile([P, 512], f32, tag="acc")
            x1b = ps_acc.tile([P, 512], f32, tag="acc")
            for k in range(16):
                nc.tensor.matmul(x1a[:], lhsT=fusedT[:, k, :],
                                 rhs=p1_w[:, k, 0:512],
                                 start=(k == 0), stop=(k == 15))
                nc.tensor.matmul(x1b[:], lhsT=fusedT[:, k, :],
                                 rhs=p1_w[:, k, 512:H],
                                 start=(k == 0), stop=(k == 15))
            xg = workA.tile([P, H], f32, tag="xg")
            if fast:
                nc.scalar.activation(out=xg[:, 0:512], in_=x1a[:], func=AF.Gelu)
                nc.scalar.activation(out=xg[:, 512:H], in_=x1b[:], func=AF.Gelu)
            else:
                nc.vector.tensor_add(out=xg[:, 0:512], in0=x1a[:],
                                     in1=p1_b[:, 0:512])
                nc.vector.tensor_add(out=xg[:, 512:H], in0=x1b[:],
                                     in1=p1_b[:, 512:H])
                nc.scalar.activation(out=xg[:], in_=xg[:], func=AF.Gelu)
            layernorm_inplace(xg[:], H, se=se)
            xgT = tpool.tile([P, 8, P], bf16, tag="xgT")
            pe_transpose(xg[:], 8, xgT)
            yield

            # out = LN(x @ p2_w.T + p2_b + emb_n) * SA
            x2_ps = ps_acc.tile([P, 512], f32, tag="acc")
            for k in range(8):
                nc.tensor.matmul(x2_ps[:], lhsT=xgT[:, k, :], rhs=p2_w[:, k, :],
                                 start=(k == 0), stop=(k == 7))
            xo = workB.tile([P, D], f32, tag="xo")
            if fast:
                nc.vector.tensor_add(out=xo[:], in0=x2_ps[:], in1=emb_n[:])
            else:
                nc.vector.tensor_add(out=xo[:], in0=x2_ps[:], in1=p2_b[:])
                nc.vector.tensor_add(out=xo[:], in0=xo[:], in1=emb_n[:])
            layernorm_inplace(xo[:], D, out_scale=SA, se=se)
            yield

            # PE-transpose xo (fp32), split into fp8 value+residual pack
            tpq = ps_t.tile([P, 4, P], f32, tag="tp")
            for blk in range(4):
                nc.tensor.matmul(
                    tpq[:, blk, :], lhsT=xo[:, blk * P:(blk + 1) * P],
                    rhs=ident[:], is_transpose=True,
                    start=(blk == 0), stop=(blk == 3), skip_group_check=True)
            nc.vector.tensor_copy(out=arpack[rt][:, 0], in_=tpq[:])
            nc.vector.tensor_sub(out=arpack[rt][:, 1], in0=tpq[:],
                                 in1=arpack[rt][:, 0])

            nc.sync.dma_start(out=agin[rt][:], in_=arpack[rt][:])
            if sim:
                nc.gpsimd.dma_start(out=agout[rt][0], in_=agin[rt][:])
            else:
                nc.gpsimd.collective_compute(
                    "AllGather", mybir.AluOpType.bypass,
                    replica_groups=[list(range(NCORES))],
                    ins=[agin[rt].opt()], outs=[agout[rt].opt()],
                )

        # rt0 sprints alone so AG0 launches as early as possible; rt1 follows
        for g in [front(0), front(1)]:
            for _ in g:
                pass

        # ---------------- lm head ----------------
        # out rows viewed as [p, parity, g2, v]; row = g2*256 + parity*128 + p
        out_r = out_ap.rearrange("(g2 two p) v -> p two g2 v", two=2, p=P)

        # one gathered-activation load per parity: [P, core, (a|r), blk, row]
        # issued on the Activation queue so waiting on the collective does not
        # head-of-line block the SP queue's weight loads / output writes.
        argall = [consts.tile([P, NCORES, 2, 4, P], fp8, tag=f"arg{r}",
                              name=f"arg{r}")
                  for r in range(RT_N)]

        def load_argall(parity):
            src = agout[parity]
            gsrc = bass.AP(tensor=src.tensor, offset=src.offset,
                           ap=[src.ap[1], src.ap[0]] + list(src.ap[2:]))
            nc.scalar.dma_start(out=argall[parity][:], in_=gsrc)

        def lm_phase(parity):
            load_argall(parity)
            for vt in range(NV):
                w = VW[vt]
                wt = lmw.tile([P, 2, 2, 2, VT], fp8, tag="wt")
                nc.sync.dma_start(out=wt[:], in_=a["wvs8"][vt])
                ls = lms.tile([P, NCORES, VT], bf16, tag=f"ls{parity}")
                vt_passes = 2 if vt < K2 else lm_passes
                for gi in range(NCORES):
                    lp = ps_lm.tile([P, VT], f32, tag="lm")
                    act = argall[parity]
                    mms = [(0, 0)]                   # (a|r, w|s)
                    if vt_passes >= 2:
                        mms.append((1, 0))
                    if vt_passes >= 3:
                        mms.append((0, 1))
                    n_mm = len(mms) * 2
                    i_mm = 0
                    for ar, ws in mms:
                        for bb in range(2):
                            nc.tensor.matmul(
                                lp[:, :w],
                                lhsT=act[:, gi, ar, 2 * bb:2 * bb + 2, :],
                                rhs=wt[:, ws, bb, :, :w],
                                start=(i_mm == 0), stop=(i_mm == n_mm - 1),
                                perf_mode=DR)
                            i_mm += 1
                    if gi % 2 == 1:
                        nc.scalar.activation(out=ls[:, gi, :w], in_=lp[:, :w],
                                             func=AF.Copy)
                    else:
                        nc.vector.tensor_copy(out=ls[:, gi, :w], in_=lp[:, :w])
                nc.sync.dma_start(
                    out=out_r[:, parity, :, vt * VT:vt * VT + w],
                    in_=ls[:, :, :w])

        lm_phase(0)   # even row tiles: ready after AG0
        lm_phase(1)   # odd row tiles: ready after AG1


# ----------------------------------------------------------------------------
# Host side
# ----------------------------------------------------------------------------

_NC_CACHE = {}
LAST_RUN = None


def get_nc(sim=False, lm_passes=3, fast=True):
    key = (sim, lm_passes, fast)
    if key not in _NC_CACHE:
        _NC_CACHE[key] = build_nc(sim, lm_passes, fast)
    return _NC_CACHE[key]


def prep_weights(inputs):
    """Host-side layout transforms. Returns (shared_map, per_core_maps)."""
    f = np.float32
    emb = np.ascontiguousarray(inputs["embedding"], dtype=f)       # [V, D]

    o_g = np.asarray(inputs["out_g"], f)
    VP = NCORES * VC
    embp = np.zeros((VP, D), dtype=f)
    embp[:V] = emb
    embw = embp * o_g[None, :] * SW                                # scaled W~

    # per-core slices are 6336 wide; pad each to the 13*512=6656 tile grid
    wpad = np.zeros((NCORES, NV * VT, D), dtype=f)
    wpad[:, :VC] = embw.reshape(NCORES, VC, D)
    # [core, vt, k, b, i, n] with d = (2b+i)*128 + k
    w_r = wpad.reshape(NCORES, NV, VT, 2, 2, P)                    # c,vt,n,b,i,k
    w_r = np.ascontiguousarray(w_r.transpose(0, 1, 5, 3, 4, 2))    # c,vt,k,b,i,n
    w8 = w_r.astype(e4m3)
    s8 = (w_r - w8.astype(f)).astype(e4m3)
    # merged [c, vt, k, (w|s), b, i, n]
    wvs8 = np.ascontiguousarray(
        np.stack([w8, s8], axis=3))                                # c,vt,k,2,b,i,n

    def t_tiles(w_t, kn, nn):
        return np.ascontiguousarray(
            w_t.reshape(kn, P, nn).transpose(1, 0, 2), dtype=np_dt(bf16))

    cp_w = t_tiles(inputs["core_proj_w"].T.astype(f), 4, C)

    h1_w = np.zeros((P, H), dtype=f)
    h1_w[:C] = inputs["h_w1"].T
    h1_w[C] = inputs["h_b1"]
    h1_w = h1_w.astype(np_dt(bf16))

    h2_w = t_tiles(inputs["h_w2"].T.astype(f), 8, C)

    up_w = np.zeros((P, D), dtype=f)
    up_w[:C] = inputs["up_w"].T
    up_w[C] = inputs["up_b"]
    up_w = up_w.astype(np_dt(bf16))

    fu_g = np.asarray(inputs["fusion_g"], f); fu_b = np.asarray(inputs["fusion_b"], f)
    p1W = inputs["p1_w"].T.astype(f)                               # [2048, 1024]
    p1Wg = fu_g[:, None] * p1W
    p1_bf = np.asarray(inputs["p1_b"], f) + fu_b @ p1W
    p1_w = np.ascontiguousarray(
        p1Wg.reshape(16, P, H).transpose(1, 0, 2), dtype=np_dt(bf16))

    pl_g = np.asarray(inputs["pln_g"], f); pl_b = np.asarray(inputs["pln_b"], f)
    p2W = inputs["p2_w"].T.astype(f)                               # [1024, 512]
    p2Wg = pl_g[:, None] * p2W
    p2_bf = np.asarray(inputs["p2_b"], f) + pl_b @ p2W
    p2_w = np.ascontiguousarray(
        p2Wg.reshape(8, P, D).transpose(1, 0, 2), dtype=np_dt(bf16))

    f0 = np.float32
    fast = (not np.any(inputs["core_proj_b"])) and (not np.any(inputs["h_b2"])) \
        and np.all(np.asarray(inputs["r_norm_g"], f0) == 1.0) \
        and (not np.any(inputs["r_norm_b"])) \
        and (not np.any(p1_bf)) and (not np.any(p2_bf))
    shared = {
        "etab": emb,
        "cp_w": cp_w,
        "cp_b": np.asarray(inputs["core_proj_b"], dtype=f),
        "h1_w": h1_w,
        "h2_w": h2_w,
        "h2_b": np.asarray(inputs["h_b2"], dtype=f),
        "up_w": up_w,
        "rn_g": np.asarray(inputs["r_norm_g"], dtype=f),
        "rn_b": np.asarray(inputs["r_norm_b"], dtype=f),
        "p1_w": p1_w,
        "p1_b": p1_bf,
        "p2_w": p2_w,
        "p2_b": p2_bf,
    }
    tok = np.asarray(inputs["token_ids"]).astype(np.int32).reshape(NCORES, BB, 1)
    per_core = []
    for c in range(NCORES):
        per_core.append({
            "tok": np.ascontiguousarray(tok[c]),
            "wvs8": np.ascontiguousarray(wvs8[c]),
        })
    return shared, per_core, fast


def run_device(inputs, trace=False, lm_passes=3):
    global LAST_RUN
    shared, per_core, fast = prep_weights(inputs)
    nc = get_nc(sim=False, lm_passes=lm_passes, fast=fast)
    in_maps = [dict(shared, **per_core[c]) for c in range(NCORES)]
    res = run_bass_kernel_spmd(nc, in_maps, list(range(NCORES)), trace=trace)
    LAST_RUN = res
    out = np.concatenate(
        [res.results[c]["out"].astype(np.float32) for c in range(NCORES)], axis=1)
    return out[:, :V] * DESCALE


def _ref_numpy(token_ids, h_prev, R_weight, embedding, core_proj_w, core_proj_b,
               h_w1, h_b1, h_w2, h_b2, up_w, up_b, r_norm_g, r_norm_b,
               fusion_g, fusion_b, p1_w, p1_b, pln_g, pln_b, p2_w, p2_b,
               out_g, out_b):
    """Exact-math fallback (only used if h_prev is nonzero)."""
    from math import erf
    f = np.float32
    ALPHA, R_DECAY, ETA_R_LOCAL, SURPRISE = 0.1, 0.999, 0.002, 1.0

    def l2n(x):
        return x / np.maximum(np.linalg.norm(x, axis=-1, keepdims=True), 1e-12)

    def ln(x, g, b):
        m = x.mean(-1, keepdims=True)
        v = x.var(-1, keepdims=True)
        return (x - m) / np.sqrt(v + 1e-5) * g + b

    emb = l2n(embedding[token_ids].astype(f))
    core_in = l2n(emb @ core_proj_w.T + core_proj_b)
    h1 = np.maximum(core_in @ h_w1.T + h_b1, 0)
    core_out = l2n(h1 @ h_w2.T + h_b2)
    core_up = l2n(core_out @ up_w.T + up_b)
    x_hat = h_prev @ R_weight
    eps = core_up - x_hat
    dR = h_prev.T @ eps / h_prev.shape[0]
    R_new = np.clip(R_DECAY * R_weight + ETA_R_LOCAL * SURPRISE * dR, -3.0, 3.0)
    temporal = h_prev @ R_new
    h_t = ln(core_up + ALPHA * temporal, r_norm_g, r_norm_b)
    fused = np.concatenate([emb, h_t, emb * h_t, emb - h_t], axis=-1)
    fused = ln(fused, fusion_g, fusion_b)
    x = fused @ p1_w.T + p1_b
    x = x * 0.5 * (1.0 + np.vectorize(erf)(x / np.sqrt(2.0)).astype(f))
    x = ln(x, pln_g, pln_b)
    x = x @ p2_w.T + p2_b
    out = ln(x + emb, out_g, out_b)
    return (out @ embedding.T).astype(f)


def kernel(**inputs):
    if np.any(np.asarray(inputs["h_prev"])):
        return _ref_numpy(**{k: np.asarray(v) for k, v in inputs.items()})
    return run_device(inputs)


if __name__ == "__main__":
    nc = build_nc(sim=True)
    print("built ok:", nc)
